# revision 38
# baseline (speedup 1.0000x reference)
"""Trainium2 Bass kernel for nn_ARM_28217935134778 (dense_cnn).

Computation (see reference): for each of the 65536 pixels of a 256x256 image,
gather a 7x7 window over 30 channels from two tensors (x: first 24 taps per
channel, x_ups: flat-tail 750 taps), feed the 1470-dim feature through a
1470 -> 2048 -> 6 MLP (ReLU in the middle), then map the 6 outputs to
(mu, scale) pairs.

Implementation: implicit-GEMM convolution, data-parallel over 8 NeuronCores
(each core takes a 32-row horizontal strip of the image).

The dominant cost under the axon tunnel is host<->device staging (~45 MB/s
for incompressible data, plus fixed per-array overhead), so the call ships
the minimum number of bytes in the minimum number of arrays:
  - ONE flat fp16 blob per core carries everything: the raw 32-row strips
    (no halo rows, no pad columns), the core's W1/W2 shards, and small
    constants. The matmuls run natively in fp16 with fp32 PSUM accumulate.
  - W1 is shipped SHARDED (184 rows per core) and AllGathered on-device over
    NeuronLink, instead of replicating the reordered matrix 8x.
  - x_ups only carries channels 14..29 (the flat tail [:, 720:] never reads
    channels 0..13).
  - Halo rows are exchanged on-device: an AllGather of each core's edge
    rows plus a one-hot-weighted selection (host-shipped selectors, zero on
    the edge cores) keeps the program SPMD-uniform.
  - The PJRT executable is compiled once and cached (_get_exec), and the
    ExternalOutput staging buffers live on-device across calls, so repeat
    calls pay only the input upload + result fetch.

Device layout per core:
  - the halo-padded strips (C,35,264)/(CU,38,264) are rebuilt in device
    DRAM: zero-fill, interior scatter, halo-row writes.
  - rhs "feature" tiles [128 feats, 512 pixels] built by shifted DMAs from
    the padded strips (one DMA covers several taps x channels).
  - W1 is host-reordered so its rows match the feature order; b1 is folded
    in via an extra constant-1.0 feature row whose W1 row equals b1.
  - Layer 1: out1[hid, pix] accumulated over 12 K-groups per 128-hid block
    (fp16 matmuls, fp32 PSUM accumulate). ReLU evict PSUM->SBUF on ACT.
  - Layer 2: out2[6, pix] accumulated over the 16 hidden chunks.
  - mu/scale transform on ACT/DVE, output stored feature-major (6, 8192);
    the host transposes/interleaves while gathering.
"""

import numpy as np

try:
    # The repeat-call cost under axon is dominated by staging + the fresh
    # jax.jit that run_bass_kernel_spmd builds per call; the persistent
    # compilation cache turns the per-call XLA re-compile into a disk hit.
    import tempfile as _tempfile
    import jax as _jax
    _jax.config.update("jax_compilation_cache_dir",
                       _tempfile.gettempdir() + "/jax_comp_cache")
    _jax.config.update("jax_persistent_cache_min_entry_size_bytes", -1)
    _jax.config.update("jax_persistent_cache_min_compile_time_secs", 0.0)
except Exception:
    pass

import concourse.bass as bass
import concourse.mybir as mybir
import concourse.tile as tile
from concourse import bacc
from concourse import bass2jax

F32 = mybir.dt.float32
F32R = mybir.dt.float32r
F16 = mybir.dt.float16

C = 30            # channels
H = W = 256
KW = 7            # window
PAD = 3
CENTER = 24       # causal taps per channel
HID = 2048
NCORES = 8
ROWS_PER_CORE = H // NCORES          # 32
PIX_PER_CORE = ROWS_PER_CORE * W     # 8192
PW = 264                             # padded row width (3 left, 5 right)
PH_X = ROWS_PER_CORE + PAD           # 35 rows: x taps only reach di 0..3
PH_U = ROWS_PER_CORE + 2 * PAD       # 38 rows for the full-window ups taps
CU0 = 14                             # first x_ups channel actually used
CU = C - CU0                         # 16 shipped x_ups channels
NPB = PIX_PER_CORE // 512            # 16 pixel blocks (2 image rows each)
NKG = 12                             # K groups (11 x 128 + 1 x 63)
KG_LAST = 63                         # 62 feature rows + 1 bias row
NM = HID // 128                      # 16 hidden blocks
W1SH = 184                           # W1 shard rows shipped per core
W1ROWS = NCORES * W1SH               # 1472 gathered rows (1471 used)
W2SH = HID * 6 // NCORES             # 1536-float W2 shard per core

# Everything ships as ONE flat fp16 tensor per core: the axon tunnel charges
# fixed overhead per transferred array, so all inputs merge into one blob.
# The strips ship RAW (own 32 rows, no halo rows, no pad columns); the
# device rebuilds the halo-padded strips: zero-fill + interior scatter +
# an AllGather halo exchange whose per-core slab is picked out with
# host-shipped one-hot weights (all-zero selectors on the edge cores keep
# the image border zero, which keeps the program SPMD-uniform).
# (f16 element offsets into the blob)
XRAW_OFF = 0
XRAW_N = C * ROWS_PER_CORE * W       # 245760
URAW_OFF = XRAW_OFF + XRAW_N
URAW_N = CU * ROWS_PER_CORE * W      # 131072
W1_OFF = URAW_OFF + URAW_N           # 376832
W1_N = W1SH * HID                    # 376832
W2_OFF = W1_OFF + W1_N               # 753664
ONES_OFF = W2_OFF + W2SH             # 755200
B3_OFF = ONES_OFF + 520              # 755720 (even: f32-bitcastable)
SEL_OFF = B3_OFF + 72                # 755792: (128,16) f32 one-hot selectors
BLOB_N = SEL_OFF + 128 * 16 * 2      # 759888 f16 elements (1.52 MB)

# halo slab sizes (f16 elements, all contiguous in the exchange buffer)
HX_N = C * PAD * W                   # 23040: x bottom rows 29..31
HUB_N = CU * PAD * W                 # 12288: ups bottom rows 29..31
HUT_N = CU * PAD * W                 # 12288: ups top rows 0..2
HC_N = HX_N + HUB_N + HUT_N          # 47616 = 128*372


def _build_runs():
    """Feature rows in our contraction order: (tensor_id, di, dj, c0, nch)."""
    runs = []
    for t in range(CENTER):                       # x: taps 0..23, all 30 ch
        runs.append((0, t // KW, t % KW, 0, C))
    for t in range(KW * KW):                      # x_ups tail
        c0 = 15 if t < 34 else 14
        runs.append((1, t // KW, t % KW, c0, C - c0))
    return runs


def _build_perm(runs):
    """Original W1 row index for each position in our feature order."""
    perm = []
    for (tid, di, dj, c0, nch) in runs:
        t = di * KW + dj
        for c in range(c0, c0 + nch):
            perm.append(c * CENTER + t if tid == 0 else c * KW * KW + t)
    assert len(perm) == 1470
    assert sorted(perm) == list(range(1470))
    return perm


def _build_pieces(runs):
    """Split runs at 128-row group boundaries, then merge consecutive taps
    (same di, channel range) into single multi-tap DMA pieces.

    Position 1408 (partition 0 of K-group 11) is reserved for the constant-1
    bias feature row, so feature positions >= 1408 shift up by one."""
    subs = []
    pos = 0
    for (tid, di, dj, c0, nch) in runs:
        left, cs = nch, c0
        while left:
            g, p = divmod(pos if pos < 1408 else pos + 1, 128)
            take = min(left, 128 - p)
            subs.append(dict(g=g, p=p, tid=tid, di=di, dj=dj, c0=cs, nch=take))
            pos += take
            cs += take
            left -= take
    assert pos == 1470
    pieces = []
    for s in subs:
        m = pieces[-1] if pieces else None
        if (m is not None and m["g"] == s["g"] and m["tid"] == s["tid"]
                and m["di"] == s["di"] and m["c0"] == s["c0"]
                and m["nch"] == s["nch"] and s["dj"] == m["dj"] + m["ntap"]
                and s["p"] == m["p"] + m["ntap"] * m["nch"]):
            m["ntap"] += 1
        else:
            pieces.append(dict(**s, ntap=1))
    return pieces


_RUNS = _build_runs()
_PERM = _build_perm(_RUNS)
_PIECES = _build_pieces(_RUNS)


def _build_nc(fbufs=2, hbufs=4, ps1bufs=7, ps2bufs=1, npb=NPB):
    nc = bacc.Bacc("TRN2", target_bir_lowering=False, debug=False,
                   num_devices=NCORES)
    # All inputs ship as ONE flat fp16 blob per core (layout in the module
    # header constants). fp16 matmul inputs keep the quantization noise
    # (~3e-4 std on the pre-clip scale logits) well inside the 2e-2
    # relative-error gate (measured 9.6e-3).
    blob = nc.dram_tensor("blob", (BLOB_N,), F16, kind="ExternalInput")
    # fp16 output: halves the result fetch through the tunnel; adds at most
    # 0.25 absolute rounding on values <= 1000.
    o = nc.dram_tensor("o", (6, PIX_PER_CORE), F16, kind="ExternalOutput")
    # per strip tensor id: (padded height, first shipped channel)
    sdim = {0: (PH_X, 0), 1: (PH_U, CU0)}

    with tile.TileContext(nc) as tc:
        with (
            tc.tile_pool(name="dpool", bufs=1, space="DRAM") as dpool,
            tc.tile_pool(name="wpool", bufs=1) as wpool,
            tc.tile_pool(name="cpool", bufs=1) as cpool,
            tc.tile_pool(name="fpool", bufs=fbufs) as fpool,
            tc.tile_pool(name="hpool", bufs=hbufs) as hpool,
            tc.tile_pool(name="spool", bufs=1) as spool,
            tc.tile_pool(name="opool", bufs=2) as opool,
            tc.tile_pool(name="ps1pool", bufs=ps1bufs, space="PSUM") as ps1pool,
            tc.tile_pool(name="ps2pool", bufs=ps2bufs, space="PSUM") as ps2pool,
        ):
            # --- W1 AllGather: shard (184, 2048) per core -> full (1472, 2048)
            # (copied to a dpool scratch tile first: EFA CC buffers need 4K
            # alignment, which a mid-blob offset can't guarantee)
            w1_cc_in = dpool.tile([W1SH, HID], F16)
            nc.sync.dma_start(w1_cc_in[:],
                              bass.AP(blob, W1_OFF, [[HID, W1SH], [1, HID]]))
            w1_full = dpool.tile([W1ROWS, HID], F16, addr_space="Shared")
            nc.gpsimd.collective_compute(
                "AllGather",
                mybir.AluOpType.bypass,
                replica_groups=[list(range(NCORES))],
                ins=[w1_cc_in[:].opt()],
                outs=[w1_full[:].opt()],
            )
            # --- W2 AllGather: 1536 floats per core -> flat (2048 x 6)
            w2_cc_in = dpool.tile([W2SH], F16)
            nc.sync.dma_start(w2_cc_in[:], bass.AP(blob, W2_OFF, [[1, W2SH]]))
            w2_full = dpool.tile([HID * 6], F16, addr_space="Shared")
            nc.gpsimd.collective_compute(
                "AllGather",
                mybir.AluOpType.bypass,
                replica_groups=[list(range(NCORES))],
                ins=[w2_cc_in[:].opt()],
                outs=[w2_full[:].opt()],
            )

            # --- rebuild the halo-padded strips from the raw (unpadded,
            # halo-less) shipped strips.
            xs_pad = dpool.tile([C, PH_X, PW], F16)
            us_pad = dpool.tile([CU, PH_U, PW], F16)
            # contiguous per-core halo contribution:
            # [x rows 29:32 | ups rows 29:32 | ups rows 0:3]
            hc = dpool.tile([HC_N], F16)
            nc.sync.dma_start(
                bass.AP(hc.tensor, hc.offset, [[PAD * W, C], [1, PAD * W]]),
                bass.AP(blob, XRAW_OFF + 29 * W,
                        [[ROWS_PER_CORE * W, C], [1, PAD * W]]))
            nc.sync.dma_start(
                bass.AP(hc.tensor, hc.offset + HX_N,
                        [[PAD * W, CU], [1, PAD * W]]),
                bass.AP(blob, URAW_OFF + 29 * W,
                        [[ROWS_PER_CORE * W, CU], [1, PAD * W]]))
            nc.sync.dma_start(
                bass.AP(hc.tensor, hc.offset + HX_N + HUB_N,
                        [[PAD * W, CU], [1, PAD * W]]),
                bass.AP(blob, URAW_OFF,
                        [[ROWS_PER_CORE * W, CU], [1, PAD * W]]))
            hall = dpool.tile([NCORES * HC_N], F16, addr_space="Shared")
            nc.gpsimd.collective_compute(
                "AllGather",
                mybir.AluOpType.bypass,
                replica_groups=[list(range(NCORES))],
                ins=[hc[:].opt()],
                outs=[hall[:].opt()],
            )

            # zero-fill both padded strips (covers pad columns and the image
            # border rows on the edge cores)
            zt = cpool.tile([30, PH_U * PW], F16)
            nc.any.memset(zt[:], 0.0)
            nc.sync.dma_start(
                bass.AP(xs_pad.tensor, xs_pad.offset,
                        [[PH_X * PW, C], [1, PH_X * PW]]),
                zt[0:C, 0:PH_X * PW])
            nc.sync.dma_start(
                bass.AP(us_pad.tensor, us_pad.offset,
                        [[PH_U * PW, CU], [1, PH_U * PW]]),
                zt[0:CU, 0:PH_U * PW])
            # scatter own rows into the interior [rows 3:35, cols 3:259]
            nc.sync.dma_start(
                bass.AP(xs_pad.tensor, xs_pad.offset + 3 * PW + 3,
                        [[PH_X * PW, C], [PW, ROWS_PER_CORE], [1, W]]),
                bass.AP(blob, XRAW_OFF,
                        [[ROWS_PER_CORE * W, C], [W, ROWS_PER_CORE], [1, W]]))
            nc.sync.dma_start(
                bass.AP(us_pad.tensor, us_pad.offset + 3 * PW + 3,
                        [[PH_U * PW, CU], [PW, ROWS_PER_CORE], [1, W]]),
                bass.AP(blob, URAW_OFF,
                        [[ROWS_PER_CORE * W, CU], [W, ROWS_PER_CORE], [1, W]]))

            # pick this core's neighbor slabs out of the gathered halo with
            # one-hot weights (selp = prev core, seln = next core; all-zero
            # on the edges) — ACT per-partition scale + DVE adds, SPMD-safe.
            sel_sb = cpool.tile([128, 16], F32)
            nc.sync.dma_start(sel_sb[:],
                              bass.AP(blob, SEL_OFF,
                                      [[32, 128], [1, 32]]).bitcast(F32))
            # partition counts chosen so the inner dim is one image row (256)
            # and the write-back DMAs balance against the strip layout
            hx_sb = cpool.tile([HX_N // W, NCORES, W], F16)       # [90, 8, 256]
            nc.sync.dma_start(
                hx_sb[:],
                bass.AP(hall.tensor, hall.offset,
                        [[W, HX_N // W], [HC_N, NCORES], [1, W]]))
            hu_sb = cpool.tile([HUB_N // W, NCORES, 2, W], F16)   # [48, 8, 2, 256]
            for t in range(2):
                nc.sync.dma_start(
                    hu_sb[:, :, t, :],
                    bass.AP(hall.tensor, hall.offset + HX_N + t * HUB_N,
                            [[W, HUB_N // W], [HC_N, NCORES], [1, W]]))

            def onehot_pick(src_j, sel_col, shape, tag):
                # sum_j sel[j] * src_j(j): ACT scale-mult then fused DVE MACs
                npart = shape[0]
                acc = spool.tile(shape, F16, tag=f"{tag}0")
                nc.scalar.activation(
                    acc[:], src_j(0),
                    mybir.ActivationFunctionType.Identity,
                    scale=sel_sb[0:npart, sel_col:sel_col + 1])
                for j in range(1, NCORES):
                    nxt = spool.tile(shape, F16, tag=f"{tag}{j}")
                    nc.vector.scalar_tensor_tensor(
                        nxt[:], src_j(j),
                        sel_sb[0:npart, sel_col + j:sel_col + j + 1], acc[:],
                        op0=mybir.AluOpType.mult,
                        op1=mybir.AluOpType.add)
                    acc = nxt
                return acc

            xtop = onehot_pick(lambda j: hx_sb[:, j, :], 0,
                               [HX_N // W, W], "hx")
            utop = onehot_pick(lambda j: hu_sb[:, j, 0, :], 0,
                               [HUB_N // W, W], "hut")
            ubot = onehot_pick(lambda j: hu_sb[:, j, 1, :], 8,
                               [HUB_N // W, W], "hub")
            # write the selected slabs into the halo rows [0:3] / [35:38],
            # cols 3:259 (flat element order matches the SBUF layout)
            nc.sync.dma_start(
                bass.AP(xs_pad.tensor, xs_pad.offset + 3,
                        [[PH_X * PW, C], [PW, PAD], [1, W]]),
                xtop[:])
            nc.sync.dma_start(
                bass.AP(us_pad.tensor, us_pad.offset + 3,
                        [[PH_U * PW, CU], [PW, PAD], [1, W]]),
                utop[:])
            nc.sync.dma_start(
                bass.AP(us_pad.tensor, us_pad.offset + 35 * PW + 3,
                        [[PH_U * PW, CU], [PW, PAD], [1, W]]),
                ubot[:])
            strip = {0: xs_pad, 1: us_pad}

            w1_sb = wpool.tile([128, NKG, HID], F16)

            # m-major lazy load: chunk m (all K-groups, one 128-wide hidden
            # block, 736KB) is issued just before pixel-block 0 consumes it,
            # so the matmul stream pipelines against the AllGather instead of
            # waiting for the whole 12MB. Gathered row g*128+p; the last
            # K-group only has 64 valid rows (1408..1471), so it loads as a
            # separate piece.
            def load_w1_chunk(m):
                base = m * 128
                nc.sync.dma_start(
                    w1_sb[:, 0:11, m * 128:(m + 1) * 128],
                    bass.AP(w1_full.tensor, w1_full.offset + base,
                            [[HID, 128], [128 * HID, 11], [1, 128]]))
                nc.sync.dma_start(
                    w1_sb[0:64, 11, m * 128:(m + 1) * 128],
                    bass.AP(w1_full.tensor,
                            w1_full.offset + 11 * 128 * HID + base,
                            [[HID, 64], [1, 128]]))
            # chunk stride padded to 16 fp16 so each lhsT slice is 32B-aligned
            w2_sb = wpool.tile([128, NM, 16], F16)
            nc.sync.dma_start(w2_sb[:, :, 0:6],
                              bass.AP(w2_full.tensor, w2_full.offset,
                                      [[6, 128], [768, NM], [1, 6]]))
            b3_sb = cpool.tile([6, 6], F32)
            nc.sync.dma_start(b3_sb[:],
                              bass.AP(blob, B3_OFF,
                                      [[12, 6], [1, 12]]).bitcast(F32))
            zb = cpool.tile([128, 1], F32)
            nc.any.memset(zb[:], 0.0)

            for pb in range(npb):
                R = 2 * pb  # first image row (strip-local) of this block
                # free layout per K-row: [2 rows, 264 cols] — a single
                # contiguous 520-element DRAM read covers both rows (the 8
                # inter-row pad columns land in [*, 0, 256:264] and are never
                # read by the matmuls).
                feat = fpool.tile([128, NKG, 2, 264], F16)
                for pc in _PIECES:
                    t = strip[pc["tid"]]
                    ph, cbase = sdim[pc["tid"]]
                    off = (t.offset + (pc["c0"] - cbase) * ph * PW
                           + (R + pc["di"]) * PW + pc["dj"])
                    src = bass.AP(
                        t.tensor,
                        off,
                        [[1, pc["ntap"]], [ph * PW, pc["nch"]], [1, 520]],
                    )
                    npart = pc["ntap"] * pc["nch"]
                    # dst: partitions p..p+npart, contiguous 520-elem span
                    # starting at [g, 0, 0]
                    dst = feat[pc["p"]:pc["p"] + npart, pc["g"], :, :]
                    dst = bass.AP(dst.tensor, dst.offset,
                                  [list(dst.ap[0]), [1, 520]])
                    nc.sync.dma_start(dst, src)
                # bias feature row: constant 1.0 (W1 row 1408 = b1)
                brow = feat[0:1, NKG - 1, :, :]
                brow = bass.AP(brow.tensor, brow.offset,
                               [list(brow.ap[0]), [1, 520]])
                nc.scalar.dma_start(brow,
                                    bass.AP(blob, ONES_OFF,
                                            [[520, 1], [1, 520]]))

                ps2 = ps2pool.tile([6, 512], F32)
                for m in range(NM):
                    if pb == 0:
                        load_w1_chunk(m)
                    ps = ps1pool.tile([128, 512], F32)
                    for g in range(NKG):
                        kg = 128 if g < NKG - 1 else KG_LAST
                        nc.tensor.matmul(
                            ps[:],
                            w1_sb[0:kg, g, m * 128:(m + 1) * 128],
                            feat[0:kg, g, :, 0:256],
                            start=(g == 0),
                            stop=(g == NKG - 1),
                        )
                    h = hpool.tile([128, 512], F16)
                    nc.scalar.activation(
                        h[:], ps[:], mybir.ActivationFunctionType.Relu,
                        bias=zb[:],
                    )
                    nc.tensor.matmul(
                        ps2[:],
                        w2_sb[:, m, 0:6],
                        h[:],
                        start=(m == 0),
                        stop=(m == NM - 1),
                        skip_group_check=True,
                    )

                # All transform ops run on partitions 0:6 with per-partition
                # scale/bias vectors; rows that don't apply get neutral values
                # (scale 0, bias 1) so every lane stays finite.
                # mu rows 0:3: (raw + b2_mu)*255 ; d rows 3:6: 1.1-(raw+b2_s)
                outm = spool.tile([6, 512], F16, tag="outm")
                nc.scalar.activation(
                    outm[:], ps2[:],
                    mybir.ActivationFunctionType.Identity,
                    bias=b3_sb[:, 1:2], scale=b3_sb[:, 0:1],
                )
                d = spool.tile([6, 512], F32, tag="d")
                nc.scalar.activation(
                    d[:], ps2[:],
                    mybir.ActivationFunctionType.Identity,
                    bias=b3_sb[:, 3:4], scale=b3_sb[:, 2:3],
                )
                r = spool.tile([6, 512], F32, tag="r")
                nc.vector.reciprocal(r[:], d[:])
                # n rows 3:6: 100*(raw + b2_s)
                n = spool.tile([6, 512], F32, tag="n")
                nc.scalar.activation(
                    n[:], ps2[:],
                    mybir.ActivationFunctionType.Identity,
                    bias=b3_sb[:, 5:6], scale=b3_sb[:, 4:5],
                )
                sc = spool.tile([6, 512], F32, tag="sc")
                nc.vector.tensor_mul(sc[:], n[:], r[:])
                scc = spool.tile([6, 512], F16, tag="scc")
                nc.vector.tensor_scalar(
                    scc[:], sc[:], 1000.0, 1e-8,
                    op0=mybir.AluOpType.min, op1=mybir.AluOpType.max,
                )
                pbs = slice(pb * 512, (pb + 1) * 512)
                nc.scalar.dma_start(o.ap()[0:3, pbs], outm[0:3, :])
                nc.scalar.dma_start(o.ap()[3:6, pbs], scc[3:6, :])

    nc.compile()
    return nc


_NC_CACHE = None


def _get_nc():
    global _NC_CACHE
    if _NC_CACHE is None:
        _NC_CACHE = _build_nc()
    return _NC_CACHE


_EXEC_CACHE = None


def _get_exec():
    """Compile the PJRT executable for the Bass kernel ONCE per process.

    run_bass_kernel_spmd builds a fresh jax.jit(shard_map(...)) every call,
    which re-traces and re-lowers (~80 ms) before the XLA disk-cache hit.
    This is the same execution path (bass_exec custom call on the 8 axon
    NeuronCores) with the compiled callable cached, and with the
    ExternalOutput staging buffers created device-side via jnp.zeros instead
    of shipping host zero arrays through the tunnel.
    """
    global _EXEC_CACHE
    if _EXEC_CACHE is not None:
        return _EXEC_CACHE

    import jax
    from jax.sharding import Mesh, PartitionSpec
    from jax.experimental.shard_map import shard_map

    nc = _get_nc()
    bass2jax.install_neuronx_cc_hook()
    partition_name = (nc.partition_id_tensor.name
                     if nc.partition_id_tensor else None)
    in_names: list[str] = []
    out_names: list[str] = []
    out_avals = []
    in_shapes = []
    for alloc in nc.m.functions[0].allocations:
        if not isinstance(alloc, mybir.MemoryLocationSet):
            continue
        name = alloc.memorylocations[0].name
        if alloc.kind == "ExternalInput":
            if name != partition_name:
                in_names.append(name)
                in_shapes.append((tuple(alloc.tensor_shape),
                                  mybir.dt.np(alloc.dtype)))
        elif alloc.kind == "ExternalOutput":
            out_names.append(name)
            out_avals.append(jax.core.ShapedArray(
                tuple(alloc.tensor_shape), mybir.dt.np(alloc.dtype)))
    n_params = len(in_names)
    bind_names = tuple(in_names) + tuple(out_names) + (
        (partition_name,) if partition_name else ())

    def _body(*args):
        # trailing args are the ExternalOutput staging buffers — the hook
        # requires every bass_exec operand to be a literal jit parameter
        operands = list(args)
        if partition_name is not None:
            operands.append(bass2jax.partition_id_tensor())
        return tuple(bass2jax._bass_exec_p.bind(
            *operands,
            out_avals=tuple(out_avals),
            in_names=bind_names,
            out_names=tuple(out_names),
            lowering_input_output_aliases=(),
            sim_require_finite=True,
            sim_require_nnan=True,
            nc=nc,
        ))

    devices = jax.devices()[:NCORES]
    mesh = Mesh(np.asarray(devices), ("core",))
    n_outs = len(out_names)
    sharded = jax.jit(shard_map(
        _body, mesh=mesh,
        in_specs=(PartitionSpec("core"),) * (n_params + n_outs),
        out_specs=(PartitionSpec("core"),) * n_outs,
        check_rep=False,
    ))
    arg_structs = [
        jax.ShapeDtypeStruct(
            (NCORES * shape[0], *shape[1:]) if shape else (NCORES,), dtype)
        for shape, dtype in in_shapes
    ] + [
        jax.ShapeDtypeStruct((NCORES * a.shape[0], *a.shape[1:]), a.dtype)
        for a in out_avals
    ]
    compiled = sharded.lower(*arg_structs).compile()
    # The staging buffers are NOT donated, so these device-resident zeros
    # survive across calls: one device_put here, zero tunnel bytes per call.
    # (o is fully overwritten by the kernel, so their value never matters.)
    from jax.sharding import NamedSharding
    zero_args = [
        jax.device_put(np.zeros((NCORES * a.shape[0], *a.shape[1:]), a.dtype),
                       NamedSharding(mesh, PartitionSpec("core")))
        for a in out_avals
    ]
    _EXEC_CACHE = (compiled, in_names, out_names, out_avals, zero_args)
    # warm up the dispatch path once (lazy PJRT/runtime init otherwise adds
    # ~100 ms to the first real call)
    try:
        warm = [np.zeros(s.shape, s.dtype) for s in arg_structs[:n_params]]
        for a in compiled(*warm, *zero_args):
            np.asarray(a)
    except Exception:
        pass
    return _EXEC_CACHE


def _run(gmap):
    """Execute on the 8 cores; gmap maps input name -> concatenated global
    array (cores stacked on axis 0). Returns {out name: (NCORES, ...) array}."""
    compiled, in_names, out_names, out_avals, zero_args = _get_exec()
    out_arrs = compiled(*[gmap[name] for name in in_names], *zero_args)
    return {
        name: np.asarray(a).reshape(NCORES, *out_avals[i].shape)
        for i, (name, a) in enumerate(zip(out_names, out_arrs))
    }


def _prep_host_inputs(x, x_ups, W1, b1, W2, b2):
    """Assemble the global input blob: (NCORES * BLOB_N,) fp16."""
    x = np.asarray(x)
    x_ups = np.asarray(x_ups)
    x16 = x[0].astype(np.float16)                  # (C, 256, 256)
    u16 = x_ups[0, CU0:].astype(np.float16)        # (CU, 256, 256)

    gblob = np.zeros((NCORES, BLOB_N), np.float16)
    xv = gblob[:, XRAW_OFF:XRAW_OFF + XRAW_N].reshape(
        NCORES, C, ROWS_PER_CORE, W)
    uv = gblob[:, URAW_OFF:URAW_OFF + URAW_N].reshape(
        NCORES, CU, ROWS_PER_CORE, W)
    for k in range(NCORES):
        r0 = k * ROWS_PER_CORE
        xv[k] = x16[:, r0:r0 + ROWS_PER_CORE]
        uv[k] = u16[:, r0:r0 + ROWS_PER_CORE]
    # one-hot halo selectors (f32, bitcast into the f16 blob), replicated
    # across the 128 SBUF partitions: cols 0:8 pick the prev core's slab,
    # cols 8:16 the next core's
    for k in range(NCORES):
        sel = gblob[k, SEL_OFF:SEL_OFF + 128 * 16 * 2].view(
            np.float32).reshape(128, 16)
        if k > 0:
            sel[:, k - 1] = 1.0
        if k < NCORES - 1:
            sel[:, 8 + k + 1] = 1.0

    # reordered W1; bias row (=b1) at position 1408; zero-pad to 1472 rows.
    # The concatenation of the 8 per-core 184-row shards IS the full
    # reordered matrix, written straight into the blob region.
    W1g = np.zeros((W1ROWS, HID), np.float16)
    W1p = np.asarray(W1)[_PERM]
    W1g[:1408] = W1p[:1408]
    W1g[1408] = np.asarray(b1)
    W1g[1409:1471] = W1p[1408:]
    gblob[:, W1_OFF:W1_OFF + W1_N] = W1g.reshape(NCORES, W1_N)
    # W2 column-reordered: [mu0 mu1 mu2 s0 s1 s2]; shards = flat slices
    W2a = np.asarray(W2).astype(np.float32)
    b2a = np.asarray(b2).astype(np.float32)
    W2f = np.ascontiguousarray(W2a[:, [0, 2, 4, 1, 3, 5]],
                               dtype=np.float16).ravel()
    gblob[:, W2_OFF:W2_OFF + W2SH] = W2f.reshape(NCORES, W2SH)
    gblob[:, ONES_OFF:ONES_OFF + 520] = np.float16(1.0)
    b3 = np.zeros((6, 6), np.float32)
    b3[0:3, 0] = 255.0                        # sA (mu scale)
    b3[0:3, 1] = b2a[[0, 2, 4]] * 255.0       # bA (mu bias)
    b3[3:6, 2] = -1.0                         # sC (d scale)
    b3[0:3, 3] = 1.0                          # bC neutral rows
    b3[3:6, 3] = 1.1 - b2a[[1, 3, 5]]         # bC (d bias)
    b3[3:6, 4] = 100.0                        # sD (n scale)
    b3[0:3, 5] = 1.0                          # bD neutral rows
    b3[3:6, 5] = 100.0 * b2a[[1, 3, 5]]       # bD (n bias)
    for k in range(NCORES):
        gblob[k, B3_OFF:B3_OFF + 72].view(np.float32)[:] = b3.ravel()

    return {"blob": gblob.reshape(-1)}


def kernel(x, x_ups, W1, b1, W2, b2):
    gmap = _prep_host_inputs(x, x_ups, W1, b1, W2, b2)
    res = _run(gmap)
    ocs = res["o"]                                                # (8, 6, 8192)
    flat = ocs.transpose(0, 2, 1).reshape(H * W, 6)               # (65536, 6)
    out = flat.reshape(H * W, 2, 3).transpose(0, 2, 1)            # (65536, 3, 2)
    return np.ascontiguousarray(out[None]).astype(np.float32)     # (1, 65536, 3, 2)



# revision 40
# speedup vs baseline: 1.0403x; 1.0403x over previous
"""Trainium2 Bass kernel for nn_ARM_28217935134778 (dense_cnn).

Computation (see reference): for each of the 65536 pixels of a 256x256 image,
gather a 7x7 window over 30 channels from two tensors (x: first 24 taps per
channel, x_ups: flat-tail 750 taps), feed the 1470-dim feature through a
1470 -> 2048 -> 6 MLP (ReLU in the middle), then map the 6 outputs to
(mu, scale) pairs.

Implementation: implicit-GEMM convolution, data-parallel over 8 NeuronCores
(each core takes a 32-row horizontal strip of the image).

The dominant cost under the axon tunnel is host<->device staging (~45 MB/s
for incompressible data, plus fixed per-array overhead), so the call ships
the minimum number of bytes in the minimum number of arrays:
  - ONE flat fp16 blob per core carries everything: the raw 32-row strips
    (no halo rows, no pad columns), the core's W1/W2 shards, and small
    constants. The matmuls run natively in fp16 with fp32 PSUM accumulate.
  - W1 is shipped SHARDED (184 rows per core) and AllGathered on-device over
    NeuronLink, instead of replicating the reordered matrix 8x.
  - x_ups only carries channels 14..29 (the flat tail [:, 720:] never reads
    channels 0..13).
  - Halo rows are exchanged on-device: an AllGather of each core's edge
    rows plus a one-hot-weighted selection (host-shipped selectors, zero on
    the edge cores) keeps the program SPMD-uniform.
  - The PJRT executable is compiled once and cached (_get_exec), and the
    ExternalOutput staging buffers live on-device across calls, so repeat
    calls pay only the input upload + result fetch.

Device layout per core:
  - the halo-padded strips (C,35,264)/(CU,38,264) are rebuilt in device
    DRAM: zero-fill, interior scatter, halo-row writes.
  - rhs "feature" tiles [128 feats, 512 pixels] built by shifted DMAs from
    the padded strips (one DMA covers several taps x channels).
  - W1 is host-reordered so its rows match the feature order; b1 is folded
    in via an extra constant-1.0 feature row whose W1 row equals b1.
  - Layer 1: out1[hid, pix] accumulated over 12 K-groups per 128-hid block
    (fp16 matmuls, fp32 PSUM accumulate). ReLU evict PSUM->SBUF on ACT.
  - Layer 2: out2[6, pix] accumulated over the 16 hidden chunks.
  - mu/scale transform on ACT/DVE, output stored feature-major (6, 8192);
    the host transposes/interleaves while gathering.
"""

import numpy as np

try:
    # The repeat-call cost under axon is dominated by staging + the fresh
    # jax.jit that run_bass_kernel_spmd builds per call; the persistent
    # compilation cache turns the per-call XLA re-compile into a disk hit.
    import tempfile as _tempfile
    import jax as _jax
    _jax.config.update("jax_compilation_cache_dir",
                       _tempfile.gettempdir() + "/jax_comp_cache")
    _jax.config.update("jax_persistent_cache_min_entry_size_bytes", -1)
    _jax.config.update("jax_persistent_cache_min_compile_time_secs", 0.0)
except Exception:
    pass

import concourse.bass as bass
import concourse.mybir as mybir
import concourse.tile as tile
from concourse import bacc
from concourse import bass2jax

F32 = mybir.dt.float32
F32R = mybir.dt.float32r
F16 = mybir.dt.float16

C = 30            # channels
H = W = 256
KW = 7            # window
PAD = 3
CENTER = 24       # causal taps per channel
HID = 2048
NCORES = 8
ROWS_PER_CORE = H // NCORES          # 32
PIX_PER_CORE = ROWS_PER_CORE * W     # 8192
PW = 264                             # padded row width (3 left, 5 right)
PH_X = ROWS_PER_CORE + PAD           # 35 rows: x taps only reach di 0..3
PH_U = ROWS_PER_CORE + 2 * PAD       # 38 rows for the full-window ups taps
CU0 = 14                             # first x_ups channel actually used
CU = C - CU0                         # 16 shipped x_ups channels
NPB = PIX_PER_CORE // 512            # 16 pixel blocks (2 image rows each)
NKG = 12                             # K groups (11 x 128 + 1 x 63)
KG_LAST = 63                         # 62 feature rows + 1 bias row
NM = HID // 128                      # 16 hidden blocks
W1SH = 184                           # W1 shard rows shipped per core
W1ROWS = NCORES * W1SH               # 1472 gathered rows (1471 used)
W2SH = HID * 6 // NCORES             # 1536-float W2 shard per core

# Everything ships as ONE flat fp16 tensor per core: the axon tunnel charges
# fixed overhead per transferred array, so all inputs merge into one blob.
# The strips ship RAW (own 32 rows, no halo rows, no pad columns); the
# device rebuilds the halo-padded strips: zero-fill + interior scatter +
# an AllGather halo exchange whose per-core slab is picked out with
# host-shipped one-hot weights (all-zero selectors on the edge cores keep
# the image border zero, which keeps the program SPMD-uniform).
# (f16 element offsets into the blob)
XRAW_OFF = 0
XRAW_N = C * ROWS_PER_CORE * W       # 245760
URAW_OFF = XRAW_OFF + XRAW_N
URAW_N = CU * ROWS_PER_CORE * W      # 131072
W1_OFF = URAW_OFF + URAW_N           # 376832
W1_N = W1SH * HID                    # 376832
W2_OFF = W1_OFF + W1_N               # 753664
ONES_OFF = W2_OFF + W2SH             # 755200
B3_OFF = ONES_OFF + 520              # 755720 (even: f32-bitcastable)
SEL_OFF = B3_OFF + 72                # 755792: (128,16) f32 one-hot selectors
BLOB_N = SEL_OFF + 128 * 16 * 2      # 759888 f16 elements (1.52 MB)

# halo slab sizes (f16 elements, all contiguous in the exchange buffer)
HX_N = C * PAD * W                   # 23040: x bottom rows 29..31
HUB_N = CU * PAD * W                 # 12288: ups bottom rows 29..31
HUT_N = CU * PAD * W                 # 12288: ups top rows 0..2
HC_N = HX_N + HUB_N + HUT_N          # 47616 = 128*372


def _build_runs():
    """Feature rows in our contraction order: (tensor_id, di, dj, c0, nch)."""
    runs = []
    for t in range(CENTER):                       # x: taps 0..23, all 30 ch
        runs.append((0, t // KW, t % KW, 0, C))
    for t in range(KW * KW):                      # x_ups tail
        c0 = 15 if t < 34 else 14
        runs.append((1, t // KW, t % KW, c0, C - c0))
    return runs


def _build_perm(runs):
    """Original W1 row index for each position in our feature order."""
    perm = []
    for (tid, di, dj, c0, nch) in runs:
        t = di * KW + dj
        for c in range(c0, c0 + nch):
            perm.append(c * CENTER + t if tid == 0 else c * KW * KW + t)
    assert len(perm) == 1470
    assert sorted(perm) == list(range(1470))
    return perm


def _build_pieces(runs):
    """Split runs at 128-row group boundaries, then merge consecutive taps
    (same di, channel range) into single multi-tap DMA pieces.

    Position 1408 (partition 0 of K-group 11) is reserved for the constant-1
    bias feature row, so feature positions >= 1408 shift up by one."""
    subs = []
    pos = 0
    for (tid, di, dj, c0, nch) in runs:
        left, cs = nch, c0
        while left:
            g, p = divmod(pos if pos < 1408 else pos + 1, 128)
            take = min(left, 128 - p)
            subs.append(dict(g=g, p=p, tid=tid, di=di, dj=dj, c0=cs, nch=take))
            pos += take
            cs += take
            left -= take
    assert pos == 1470
    pieces = []
    for s in subs:
        m = pieces[-1] if pieces else None
        if (m is not None and m["g"] == s["g"] and m["tid"] == s["tid"]
                and m["di"] == s["di"] and m["c0"] == s["c0"]
                and m["nch"] == s["nch"] and s["dj"] == m["dj"] + m["ntap"]
                and s["p"] == m["p"] + m["ntap"] * m["nch"]):
            m["ntap"] += 1
        else:
            pieces.append(dict(**s, ntap=1))
    return pieces


_RUNS = _build_runs()
_PERM = _build_perm(_RUNS)
_PIECES = _build_pieces(_RUNS)


def _build_nc(fbufs=2, hbufs=4, ps1bufs=7, ps2bufs=1, npb=NPB):
    nc = bacc.Bacc("TRN2", target_bir_lowering=False, debug=False,
                   num_devices=NCORES)
    # All inputs ship as ONE flat fp16 blob per core (layout in the module
    # header constants). fp16 matmul inputs keep the quantization noise
    # (~3e-4 std on the pre-clip scale logits) well inside the 2e-2
    # relative-error gate (measured 9.6e-3).
    blob = nc.dram_tensor("blob", (BLOB_N,), F16, kind="ExternalInput")
    # fp16 output: halves the result fetch through the tunnel; adds at most
    # 0.25 absolute rounding on values <= 1000.
    o = nc.dram_tensor("o", (6, PIX_PER_CORE), F16, kind="ExternalOutput")
    # per strip tensor id: (padded height, first shipped channel)
    sdim = {0: (PH_X, 0), 1: (PH_U, CU0)}

    with tile.TileContext(nc) as tc:
        with (
            tc.tile_pool(name="dpool", bufs=1, space="DRAM") as dpool,
            tc.tile_pool(name="wpool", bufs=1) as wpool,
            tc.tile_pool(name="cpool", bufs=1) as cpool,
            tc.tile_pool(name="fpool", bufs=fbufs) as fpool,
            tc.tile_pool(name="hpool", bufs=hbufs) as hpool,
            tc.tile_pool(name="spool", bufs=1) as spool,
            tc.tile_pool(name="opool", bufs=2) as opool,
            tc.tile_pool(name="ps1pool", bufs=ps1bufs, space="PSUM") as ps1pool,
            tc.tile_pool(name="ps2pool", bufs=ps2bufs, space="PSUM") as ps2pool,
        ):
            # --- W1 AllGather: shard (184, 2048) per core -> full (1472, 2048)
            # (copied to a dpool scratch tile first: EFA CC buffers need 4K
            # alignment, which a mid-blob offset can't guarantee)
            w1_cc_in = dpool.tile([W1SH, HID], F16)
            nc.sync.dma_start(w1_cc_in[:],
                              bass.AP(blob, W1_OFF, [[HID, W1SH], [1, HID]]))
            w1_full = dpool.tile([W1ROWS, HID], F16, addr_space="Shared")
            nc.gpsimd.collective_compute(
                "AllGather",
                mybir.AluOpType.bypass,
                replica_groups=[list(range(NCORES))],
                ins=[w1_cc_in[:].opt()],
                outs=[w1_full[:].opt()],
            )
            # --- W2 AllGather: 1536 floats per core -> flat (2048 x 6)
            w2_cc_in = dpool.tile([W2SH], F16)
            nc.sync.dma_start(w2_cc_in[:], bass.AP(blob, W2_OFF, [[1, W2SH]]))
            w2_full = dpool.tile([HID * 6], F16, addr_space="Shared")
            nc.gpsimd.collective_compute(
                "AllGather",
                mybir.AluOpType.bypass,
                replica_groups=[list(range(NCORES))],
                ins=[w2_cc_in[:].opt()],
                outs=[w2_full[:].opt()],
            )

            # --- rebuild the halo-padded strips from the raw (unpadded,
            # halo-less) shipped strips.
            xs_pad = dpool.tile([C, PH_X, PW], F16)
            us_pad = dpool.tile([CU, PH_U, PW], F16)
            # contiguous per-core halo contribution:
            # [x rows 29:32 | ups rows 29:32 | ups rows 0:3]
            hc = dpool.tile([HC_N], F16)
            nc.sync.dma_start(
                bass.AP(hc.tensor, hc.offset, [[PAD * W, C], [1, PAD * W]]),
                bass.AP(blob, XRAW_OFF + 29 * W,
                        [[ROWS_PER_CORE * W, C], [1, PAD * W]]))
            nc.sync.dma_start(
                bass.AP(hc.tensor, hc.offset + HX_N,
                        [[PAD * W, CU], [1, PAD * W]]),
                bass.AP(blob, URAW_OFF + 29 * W,
                        [[ROWS_PER_CORE * W, CU], [1, PAD * W]]))
            nc.sync.dma_start(
                bass.AP(hc.tensor, hc.offset + HX_N + HUB_N,
                        [[PAD * W, CU], [1, PAD * W]]),
                bass.AP(blob, URAW_OFF,
                        [[ROWS_PER_CORE * W, CU], [1, PAD * W]]))
            hall = dpool.tile([NCORES * HC_N], F16, addr_space="Shared")
            nc.gpsimd.collective_compute(
                "AllGather",
                mybir.AluOpType.bypass,
                replica_groups=[list(range(NCORES))],
                ins=[hc[:].opt()],
                outs=[hall[:].opt()],
            )

            # zero-fill both padded strips (covers pad columns and the image
            # border rows on the edge cores)
            zt = cpool.tile([30, PH_U * PW], F16)
            nc.any.memset(zt[:], 0.0)
            nc.sync.dma_start(
                bass.AP(xs_pad.tensor, xs_pad.offset,
                        [[PH_X * PW, C], [1, PH_X * PW]]),
                zt[0:C, 0:PH_X * PW])
            nc.sync.dma_start(
                bass.AP(us_pad.tensor, us_pad.offset,
                        [[PH_U * PW, CU], [1, PH_U * PW]]),
                zt[0:CU, 0:PH_U * PW])
            # scatter own rows into the interior [rows 3:35, cols 3:259]
            nc.sync.dma_start(
                bass.AP(xs_pad.tensor, xs_pad.offset + 3 * PW + 3,
                        [[PH_X * PW, C], [PW, ROWS_PER_CORE], [1, W]]),
                bass.AP(blob, XRAW_OFF,
                        [[ROWS_PER_CORE * W, C], [W, ROWS_PER_CORE], [1, W]]))
            nc.sync.dma_start(
                bass.AP(us_pad.tensor, us_pad.offset + 3 * PW + 3,
                        [[PH_U * PW, CU], [PW, ROWS_PER_CORE], [1, W]]),
                bass.AP(blob, URAW_OFF,
                        [[ROWS_PER_CORE * W, CU], [W, ROWS_PER_CORE], [1, W]]))

            # pick this core's neighbor slabs out of the gathered halo with
            # one-hot weights (selp = prev core, seln = next core; all-zero
            # on the edges) — ACT per-partition scale + DVE adds, SPMD-safe.
            sel_sb = cpool.tile([128, 16], F32)
            nc.sync.dma_start(sel_sb[:],
                              bass.AP(blob, SEL_OFF,
                                      [[32, 128], [1, 32]]).bitcast(F32))
            # partition counts chosen so the inner dim is one image row (256)
            # and the write-back DMAs balance against the strip layout
            hx_sb = cpool.tile([HX_N // W, NCORES, W], F16)       # [90, 8, 256]
            nc.sync.dma_start(
                hx_sb[:],
                bass.AP(hall.tensor, hall.offset,
                        [[W, HX_N // W], [HC_N, NCORES], [1, W]]))
            hu_sb = cpool.tile([HUB_N // W, NCORES, 2, W], F16)   # [48, 8, 2, 256]
            for t in range(2):
                nc.sync.dma_start(
                    hu_sb[:, :, t, :],
                    bass.AP(hall.tensor, hall.offset + HX_N + t * HUB_N,
                            [[W, HUB_N // W], [HC_N, NCORES], [1, W]]))

            def onehot_pick(src_j, sel_col, shape, tag):
                # sum_j sel[j] * src_j(j): ACT scale-mult then fused DVE MACs
                npart = shape[0]
                acc = spool.tile(shape, F16, tag=f"{tag}0")
                nc.scalar.activation(
                    acc[:], src_j(0),
                    mybir.ActivationFunctionType.Identity,
                    scale=sel_sb[0:npart, sel_col:sel_col + 1])
                for j in range(1, NCORES):
                    nxt = spool.tile(shape, F16, tag=f"{tag}{j}")
                    nc.vector.scalar_tensor_tensor(
                        nxt[:], src_j(j),
                        sel_sb[0:npart, sel_col + j:sel_col + j + 1], acc[:],
                        op0=mybir.AluOpType.mult,
                        op1=mybir.AluOpType.add)
                    acc = nxt
                return acc

            xtop = onehot_pick(lambda j: hx_sb[:, j, :], 0,
                               [HX_N // W, W], "hx")
            utop = onehot_pick(lambda j: hu_sb[:, j, 0, :], 0,
                               [HUB_N // W, W], "hut")
            ubot = onehot_pick(lambda j: hu_sb[:, j, 1, :], 8,
                               [HUB_N // W, W], "hub")
            # write the selected slabs into the halo rows [0:3] / [35:38],
            # cols 3:259 (flat element order matches the SBUF layout)
            nc.sync.dma_start(
                bass.AP(xs_pad.tensor, xs_pad.offset + 3,
                        [[PH_X * PW, C], [PW, PAD], [1, W]]),
                xtop[:])
            nc.sync.dma_start(
                bass.AP(us_pad.tensor, us_pad.offset + 3,
                        [[PH_U * PW, CU], [PW, PAD], [1, W]]),
                utop[:])
            nc.sync.dma_start(
                bass.AP(us_pad.tensor, us_pad.offset + 35 * PW + 3,
                        [[PH_U * PW, CU], [PW, PAD], [1, W]]),
                ubot[:])
            strip = {0: xs_pad, 1: us_pad}

            w1_sb = wpool.tile([128, NKG, HID], F16)

            # m-major lazy load: chunk m (all K-groups, one 128-wide hidden
            # block, 736KB) is issued just before pixel-block 0 consumes it,
            # so the matmul stream pipelines against the AllGather instead of
            # waiting for the whole 12MB. Gathered row g*128+p; the last
            # K-group only has 64 valid rows (1408..1471), so it loads as a
            # separate piece.
            def load_w1_chunk(m):
                base = m * 128
                nc.sync.dma_start(
                    w1_sb[:, 0:11, m * 128:(m + 1) * 128],
                    bass.AP(w1_full.tensor, w1_full.offset + base,
                            [[HID, 128], [128 * HID, 11], [1, 128]]))
                nc.sync.dma_start(
                    w1_sb[0:64, 11, m * 128:(m + 1) * 128],
                    bass.AP(w1_full.tensor,
                            w1_full.offset + 11 * 128 * HID + base,
                            [[HID, 64], [1, 128]]))
            # chunk stride padded to 16 fp16 so each lhsT slice is 32B-aligned
            w2_sb = wpool.tile([128, NM, 16], F16)
            nc.sync.dma_start(w2_sb[:, :, 0:6],
                              bass.AP(w2_full.tensor, w2_full.offset,
                                      [[6, 128], [768, NM], [1, 6]]))
            b3_sb = cpool.tile([6, 6], F32)
            nc.sync.dma_start(b3_sb[:],
                              bass.AP(blob, B3_OFF,
                                      [[12, 6], [1, 12]]).bitcast(F32))
            zb = cpool.tile([128, 1], F32)
            nc.any.memset(zb[:], 0.0)

            for pb in range(npb):
                R = 2 * pb  # first image row (strip-local) of this block
                # free layout per K-row: [2 rows, 264 cols] — a single
                # contiguous 520-element DRAM read covers both rows (the 8
                # inter-row pad columns land in [*, 0, 256:264] and are never
                # read by the matmuls).
                feat = fpool.tile([128, NKG, 2, 264], F16)
                for pc in _PIECES:
                    t = strip[pc["tid"]]
                    ph, cbase = sdim[pc["tid"]]
                    off = (t.offset + (pc["c0"] - cbase) * ph * PW
                           + (R + pc["di"]) * PW + pc["dj"])
                    src = bass.AP(
                        t.tensor,
                        off,
                        [[1, pc["ntap"]], [ph * PW, pc["nch"]], [1, 520]],
                    )
                    npart = pc["ntap"] * pc["nch"]
                    # dst: partitions p..p+npart, contiguous 520-elem span
                    # starting at [g, 0, 0]
                    dst = feat[pc["p"]:pc["p"] + npart, pc["g"], :, :]
                    dst = bass.AP(dst.tensor, dst.offset,
                                  [list(dst.ap[0]), [1, 520]])
                    nc.sync.dma_start(dst, src)
                # bias feature row: constant 1.0 (W1 row 1408 = b1)
                brow = feat[0:1, NKG - 1, :, :]
                brow = bass.AP(brow.tensor, brow.offset,
                               [list(brow.ap[0]), [1, 520]])
                nc.scalar.dma_start(brow,
                                    bass.AP(blob, ONES_OFF,
                                            [[520, 1], [1, 520]]))

                ps2 = ps2pool.tile([6, 512], F32)
                for m in range(NM):
                    if pb == 0:
                        load_w1_chunk(m)
                    ps = ps1pool.tile([128, 512], F32)
                    for g in range(NKG):
                        kg = 128 if g < NKG - 1 else KG_LAST
                        nc.tensor.matmul(
                            ps[:],
                            w1_sb[0:kg, g, m * 128:(m + 1) * 128],
                            feat[0:kg, g, :, 0:256],
                            start=(g == 0),
                            stop=(g == NKG - 1),
                        )
                    h = hpool.tile([128, 512], F16)
                    nc.scalar.activation(
                        h[:], ps[:], mybir.ActivationFunctionType.Relu,
                        bias=zb[:],
                    )
                    nc.tensor.matmul(
                        ps2[:],
                        w2_sb[:, m, 0:6],
                        h[:],
                        start=(m == 0),
                        stop=(m == NM - 1),
                        skip_group_check=True,
                    )

                # All transform ops run on partitions 0:6 with per-partition
                # scale/bias vectors; rows that don't apply get neutral values
                # (scale 0, bias 1) so every lane stays finite.
                # mu rows 0:3: (raw + b2_mu)*255 ; d rows 3:6: 1.1-(raw+b2_s)
                outm = spool.tile([6, 512], F16, tag="outm")
                nc.scalar.activation(
                    outm[:], ps2[:],
                    mybir.ActivationFunctionType.Identity,
                    bias=b3_sb[:, 1:2], scale=b3_sb[:, 0:1],
                )
                d = spool.tile([6, 512], F32, tag="d")
                nc.scalar.activation(
                    d[:], ps2[:],
                    mybir.ActivationFunctionType.Identity,
                    bias=b3_sb[:, 3:4], scale=b3_sb[:, 2:3],
                )
                r = spool.tile([6, 512], F32, tag="r")
                nc.vector.reciprocal(r[:], d[:])
                # n rows 3:6: 100*(raw + b2_s)
                n = spool.tile([6, 512], F32, tag="n")
                nc.scalar.activation(
                    n[:], ps2[:],
                    mybir.ActivationFunctionType.Identity,
                    bias=b3_sb[:, 5:6], scale=b3_sb[:, 4:5],
                )
                sc = spool.tile([6, 512], F32, tag="sc")
                nc.vector.tensor_mul(sc[:], n[:], r[:])
                scc = spool.tile([6, 512], F16, tag="scc")
                nc.vector.tensor_scalar(
                    scc[:], sc[:], 1000.0, 1e-8,
                    op0=mybir.AluOpType.min, op1=mybir.AluOpType.max,
                )
                pbs = slice(pb * 512, (pb + 1) * 512)
                nc.scalar.dma_start(o.ap()[0:3, pbs], outm[0:3, :])
                nc.scalar.dma_start(o.ap()[3:6, pbs], scc[3:6, :])

    nc.compile()
    return nc


_NC_CACHE = None


def _get_nc():
    global _NC_CACHE
    if _NC_CACHE is None:
        _NC_CACHE = _build_nc()
    return _NC_CACHE


_EXEC_CACHE = None


def _get_exec():
    """Compile the PJRT executable for the Bass kernel ONCE per process.

    run_bass_kernel_spmd builds a fresh jax.jit(shard_map(...)) every call,
    which re-traces and re-lowers (~80 ms) before the XLA disk-cache hit.
    This is the same execution path (bass_exec custom call on the 8 axon
    NeuronCores) with the compiled callable cached, and with the
    ExternalOutput staging buffers kept device-resident (they are not
    donated, so one device_put serves every call) instead of shipping host
    zero arrays through the tunnel each time.
    """
    global _EXEC_CACHE
    if _EXEC_CACHE is not None:
        return _EXEC_CACHE

    import jax
    from jax.sharding import Mesh, PartitionSpec
    from jax.experimental.shard_map import shard_map

    nc = _get_nc()
    bass2jax.install_neuronx_cc_hook()
    partition_name = (nc.partition_id_tensor.name
                     if nc.partition_id_tensor else None)
    in_names: list[str] = []
    out_names: list[str] = []
    out_avals = []
    in_shapes = []
    for alloc in nc.m.functions[0].allocations:
        if not isinstance(alloc, mybir.MemoryLocationSet):
            continue
        name = alloc.memorylocations[0].name
        if alloc.kind == "ExternalInput":
            if name != partition_name:
                in_names.append(name)
                in_shapes.append((tuple(alloc.tensor_shape),
                                  mybir.dt.np(alloc.dtype)))
        elif alloc.kind == "ExternalOutput":
            out_names.append(name)
            out_avals.append(jax.core.ShapedArray(
                tuple(alloc.tensor_shape), mybir.dt.np(alloc.dtype)))
    n_params = len(in_names)
    bind_names = tuple(in_names) + tuple(out_names) + (
        (partition_name,) if partition_name else ())

    def _body(*args):
        # trailing args are the ExternalOutput staging buffers — the hook
        # requires every bass_exec operand to be a literal jit parameter
        operands = list(args)
        if partition_name is not None:
            operands.append(bass2jax.partition_id_tensor())
        return tuple(bass2jax._bass_exec_p.bind(
            *operands,
            out_avals=tuple(out_avals),
            in_names=bind_names,
            out_names=tuple(out_names),
            lowering_input_output_aliases=(),
            sim_require_finite=True,
            sim_require_nnan=True,
            nc=nc,
        ))

    devices = jax.devices()[:NCORES]
    mesh = Mesh(np.asarray(devices), ("core",))
    n_outs = len(out_names)
    sharded = jax.jit(shard_map(
        _body, mesh=mesh,
        in_specs=(PartitionSpec("core"),) * (n_params + n_outs),
        out_specs=(PartitionSpec("core"),) * n_outs,
        check_rep=False,
    ))
    arg_structs = [
        jax.ShapeDtypeStruct(
            (NCORES * shape[0], *shape[1:]) if shape else (NCORES,), dtype)
        for shape, dtype in in_shapes
    ] + [
        jax.ShapeDtypeStruct((NCORES * a.shape[0], *a.shape[1:]), a.dtype)
        for a in out_avals
    ]
    compiled = sharded.lower(*arg_structs).compile()
    # The staging buffers are NOT donated, so these device-resident zeros
    # survive across calls: one device_put here, zero tunnel bytes per call.
    # (o is fully overwritten by the kernel, so their value never matters.)
    from jax.sharding import NamedSharding
    zero_args = [
        jax.device_put(np.zeros((NCORES * a.shape[0], *a.shape[1:]), a.dtype),
                       NamedSharding(mesh, PartitionSpec("core")))
        for a in out_avals
    ]
    _EXEC_CACHE = (compiled, in_names, out_names, out_avals, zero_args)
    # warm up the dispatch path once with incompressible data (lazy
    # PJRT/tunnel init otherwise adds ~100 ms to the first real call)
    try:
        rng = np.random.default_rng(0)
        warm = [
            rng.integers(0, 60, size=s.shape, dtype=np.uint8).view(
                np.dtype(s.dtype)) if np.dtype(s.dtype).itemsize == 1
            else rng.integers(0, 15360, size=s.shape,
                              dtype=np.uint16).view(np.dtype(s.dtype))
            for s in arg_structs[:n_params]
        ]
        for a in compiled(*warm, *zero_args):
            np.asarray(a)
    except Exception:
        pass
    return _EXEC_CACHE


def _run(gmap):
    """Execute on the 8 cores; gmap maps input name -> concatenated global
    array (cores stacked on axis 0). Returns {out name: (NCORES, ...) array}."""
    compiled, in_names, out_names, out_avals, zero_args = _get_exec()
    out_arrs = compiled(*[gmap[name] for name in in_names], *zero_args)
    return {
        name: np.asarray(a).reshape(NCORES, *out_avals[i].shape)
        for i, (name, a) in enumerate(zip(out_names, out_arrs))
    }


def _prep_host_inputs(x, x_ups, W1, b1, W2, b2):
    """Assemble the global input blob: (NCORES * BLOB_N,) fp16."""
    x = np.asarray(x)
    x_ups = np.asarray(x_ups)
    x16 = x[0].astype(np.float16)                  # (C, 256, 256)
    u16 = x_ups[0, CU0:].astype(np.float16)        # (CU, 256, 256)

    gblob = np.zeros((NCORES, BLOB_N), np.float16)
    xv = gblob[:, XRAW_OFF:XRAW_OFF + XRAW_N].reshape(
        NCORES, C, ROWS_PER_CORE, W)
    uv = gblob[:, URAW_OFF:URAW_OFF + URAW_N].reshape(
        NCORES, CU, ROWS_PER_CORE, W)
    for k in range(NCORES):
        r0 = k * ROWS_PER_CORE
        xv[k] = x16[:, r0:r0 + ROWS_PER_CORE]
        uv[k] = u16[:, r0:r0 + ROWS_PER_CORE]
    # one-hot halo selectors (f32, bitcast into the f16 blob), replicated
    # across the 128 SBUF partitions: cols 0:8 pick the prev core's slab,
    # cols 8:16 the next core's
    for k in range(NCORES):
        sel = gblob[k, SEL_OFF:SEL_OFF + 128 * 16 * 2].view(
            np.float32).reshape(128, 16)
        if k > 0:
            sel[:, k - 1] = 1.0
        if k < NCORES - 1:
            sel[:, 8 + k + 1] = 1.0

    # reordered W1; bias row (=b1) at position 1408; zero-pad to 1472 rows.
    # The concatenation of the 8 per-core 184-row shards IS the full
    # reordered matrix, written straight into the blob region.
    W1g = np.zeros((W1ROWS, HID), np.float16)
    W1p = np.asarray(W1)[_PERM]
    W1g[:1408] = W1p[:1408]
    W1g[1408] = np.asarray(b1)
    W1g[1409:1471] = W1p[1408:]
    gblob[:, W1_OFF:W1_OFF + W1_N] = W1g.reshape(NCORES, W1_N)
    # W2 column-reordered: [mu0 mu1 mu2 s0 s1 s2]; shards = flat slices
    W2a = np.asarray(W2).astype(np.float32)
    b2a = np.asarray(b2).astype(np.float32)
    W2f = np.ascontiguousarray(W2a[:, [0, 2, 4, 1, 3, 5]],
                               dtype=np.float16).ravel()
    gblob[:, W2_OFF:W2_OFF + W2SH] = W2f.reshape(NCORES, W2SH)
    gblob[:, ONES_OFF:ONES_OFF + 520] = np.float16(1.0)
    b3 = np.zeros((6, 6), np.float32)
    b3[0:3, 0] = 255.0                        # sA (mu scale)
    b3[0:3, 1] = b2a[[0, 2, 4]] * 255.0       # bA (mu bias)
    b3[3:6, 2] = -1.0                         # sC (d scale)
    b3[0:3, 3] = 1.0                          # bC neutral rows
    b3[3:6, 3] = 1.1 - b2a[[1, 3, 5]]         # bC (d bias)
    b3[3:6, 4] = 100.0                        # sD (n scale)
    b3[0:3, 5] = 1.0                          # bD neutral rows
    b3[3:6, 5] = 100.0 * b2a[[1, 3, 5]]       # bD (n bias)
    for k in range(NCORES):
        gblob[k, B3_OFF:B3_OFF + 72].view(np.float32)[:] = b3.ravel()

    return {"blob": gblob.reshape(-1)}


def kernel(x, x_ups, W1, b1, W2, b2):
    gmap = _prep_host_inputs(x, x_ups, W1, b1, W2, b2)
    res = _run(gmap)
    ocs = res["o"]                                                # (8, 6, 8192)
    flat = ocs.transpose(0, 2, 1).reshape(H * W, 6)               # (65536, 6)
    out = flat.reshape(H * W, 2, 3).transpose(0, 2, 1)            # (65536, 3, 2)
    return np.ascontiguousarray(out[None]).astype(np.float32)     # (1, 65536, 3, 2)



# revision 41
# speedup vs baseline: 1.0631x; 1.0220x over previous
"""Trainium2 Bass kernel for nn_ARM_28217935134778 (dense_cnn).

Computation (see reference): for each of the 65536 pixels of a 256x256 image,
gather a 7x7 window over 30 channels from two tensors (x: first 24 taps per
channel, x_ups: flat-tail 750 taps), feed the 1470-dim feature through a
1470 -> 2048 -> 6 MLP (ReLU in the middle), then map the 6 outputs to
(mu, scale) pairs.

Implementation: implicit-GEMM convolution, data-parallel over 8 NeuronCores
(each core takes a 32-row horizontal strip of the image).

The dominant cost under the axon tunnel is host<->device staging (~45 MB/s
for incompressible data, plus fixed per-array overhead), so the call ships
the minimum number of bytes in the minimum number of arrays:
  - ONE flat fp16 blob per core carries everything: the raw 32-row strips
    (no halo rows, no pad columns), the core's W1/W2 shards, and small
    constants. The matmuls run natively in fp16 with fp32 PSUM accumulate.
  - W1 is shipped SHARDED (184 rows per core) and AllGathered on-device over
    NeuronLink, instead of replicating the reordered matrix 8x.
  - x_ups only carries channels 14..29 (the flat tail [:, 720:] never reads
    channels 0..13).
  - Halo rows are exchanged on-device: an AllGather of each core's edge
    rows plus a one-hot-weighted selection (host-shipped selectors, zero on
    the edge cores) keeps the program SPMD-uniform.
  - The PJRT executable is compiled once and cached (_get_exec), and the
    ExternalOutput staging buffers live on-device across calls, so repeat
    calls pay only the input upload + result fetch.

Device layout per core:
  - the halo-padded strips (C,35,264)/(CU,38,264) are rebuilt in device
    DRAM: zero-fill, interior scatter, halo-row writes.
  - rhs "feature" tiles [128 feats, 512 pixels] built by shifted DMAs from
    the padded strips (one DMA covers several taps x channels).
  - W1 is host-reordered so its rows match the feature order; b1 is folded
    in via an extra constant-1.0 feature row whose W1 row equals b1.
  - Layer 1: out1[hid, pix] accumulated over 12 K-groups per 128-hid block
    (fp16 matmuls, fp32 PSUM accumulate). ReLU evict PSUM->SBUF on ACT.
  - Layer 2: out2[6, pix] accumulated over the 16 hidden chunks.
  - mu/scale transform on ACT/DVE, output stored feature-major (6, 8192);
    the host transposes/interleaves while gathering.
"""

import numpy as np

try:
    # The repeat-call cost under axon is dominated by staging + the fresh
    # jax.jit that run_bass_kernel_spmd builds per call; the persistent
    # compilation cache turns the per-call XLA re-compile into a disk hit.
    import tempfile as _tempfile
    import jax as _jax
    _jax.config.update("jax_compilation_cache_dir",
                       _tempfile.gettempdir() + "/jax_comp_cache")
    _jax.config.update("jax_persistent_cache_min_entry_size_bytes", -1)
    _jax.config.update("jax_persistent_cache_min_compile_time_secs", 0.0)
except Exception:
    pass

import concourse.bass as bass
import concourse.mybir as mybir
import concourse.tile as tile
from concourse import bacc
from concourse import bass2jax

F32 = mybir.dt.float32
F32R = mybir.dt.float32r
F16 = mybir.dt.float16

C = 30            # channels
H = W = 256
KW = 7            # window
PAD = 3
CENTER = 24       # causal taps per channel
HID = 2048
NCORES = 8
ROWS_PER_CORE = H // NCORES          # 32
PIX_PER_CORE = ROWS_PER_CORE * W     # 8192
PW = 264                             # padded row width (3 left, 5 right)
PH_X = ROWS_PER_CORE + PAD           # 35 rows: x taps only reach di 0..3
PH_U = ROWS_PER_CORE + 2 * PAD       # 38 rows for the full-window ups taps
CU0 = 14                             # first x_ups channel actually used
CU = C - CU0                         # 16 shipped x_ups channels
NPB = PIX_PER_CORE // 512            # 16 pixel blocks (2 image rows each)
NKG = 12                             # K groups (11 x 128 + 1 x 63)
KG_LAST = 63                         # 62 feature rows + 1 bias row
NM = HID // 128                      # 16 hidden blocks
W1SH = 184                           # W1 shard rows shipped per core
W1ROWS = NCORES * W1SH               # 1472 gathered rows (1471 used)
W2SH = HID * 6 // NCORES             # 1536-float W2 shard per core

# Everything ships as ONE flat fp16 tensor per core: the axon tunnel charges
# fixed overhead per transferred array, so all inputs merge into one blob.
# The strips ship RAW (own 32 rows, no halo rows, no pad columns); the
# device rebuilds the halo-padded strips: zero-fill + interior scatter +
# an AllGather halo exchange whose per-core slab is picked out with
# host-shipped one-hot weights (all-zero selectors on the edge cores keep
# the image border zero, which keeps the program SPMD-uniform).
# (f16 element offsets into the blob)
XRAW_OFF = 0
XRAW_N = C * ROWS_PER_CORE * W       # 245760
URAW_OFF = XRAW_OFF + XRAW_N
URAW_N = CU * ROWS_PER_CORE * W      # 131072
W1_OFF = URAW_OFF + URAW_N           # 376832
W1_N = W1SH * HID                    # 376832
W2_OFF = W1_OFF + W1_N               # 753664
ONES_OFF = W2_OFF + W2SH             # 755200
B3_OFF = ONES_OFF + 520              # 755720 (even: f32-bitcastable)
SEL_OFF = B3_OFF + 72                # 755792: (128,16) f32 one-hot selectors
BLOB_N = SEL_OFF + 128 * 16 * 2      # 759888 f16 elements (1.52 MB)

# halo slab sizes (f16 elements, all contiguous in the exchange buffer)
HX_N = C * PAD * W                   # 23040: x bottom rows 29..31
HUB_N = CU * PAD * W                 # 12288: ups bottom rows 29..31
HUT_N = CU * PAD * W                 # 12288: ups top rows 0..2
HC_N = HX_N + HUB_N + HUT_N          # 47616 = 128*372


def _build_runs():
    """Feature rows in our contraction order: (tensor_id, di, dj, c0, nch)."""
    runs = []
    for t in range(CENTER):                       # x: taps 0..23, all 30 ch
        runs.append((0, t // KW, t % KW, 0, C))
    for t in range(KW * KW):                      # x_ups tail
        c0 = 15 if t < 34 else 14
        runs.append((1, t // KW, t % KW, c0, C - c0))
    return runs


def _build_perm(runs):
    """Original W1 row index for each position in our feature order."""
    perm = []
    for (tid, di, dj, c0, nch) in runs:
        t = di * KW + dj
        for c in range(c0, c0 + nch):
            perm.append(c * CENTER + t if tid == 0 else c * KW * KW + t)
    assert len(perm) == 1470
    assert sorted(perm) == list(range(1470))
    return perm


def _build_pieces(runs):
    """Split runs at 128-row group boundaries, then merge consecutive taps
    (same di, channel range) into single multi-tap DMA pieces.

    Position 1408 (partition 0 of K-group 11) is reserved for the constant-1
    bias feature row, so feature positions >= 1408 shift up by one."""
    subs = []
    pos = 0
    for (tid, di, dj, c0, nch) in runs:
        left, cs = nch, c0
        while left:
            g, p = divmod(pos if pos < 1408 else pos + 1, 128)
            take = min(left, 128 - p)
            subs.append(dict(g=g, p=p, tid=tid, di=di, dj=dj, c0=cs, nch=take))
            pos += take
            cs += take
            left -= take
    assert pos == 1470
    pieces = []
    for s in subs:
        m = pieces[-1] if pieces else None
        if (m is not None and m["g"] == s["g"] and m["tid"] == s["tid"]
                and m["di"] == s["di"] and m["c0"] == s["c0"]
                and m["nch"] == s["nch"] and s["dj"] == m["dj"] + m["ntap"]
                and s["p"] == m["p"] + m["ntap"] * m["nch"]):
            m["ntap"] += 1
        else:
            pieces.append(dict(**s, ntap=1))
    return pieces


_RUNS = _build_runs()
_PERM = _build_perm(_RUNS)
_PIECES = _build_pieces(_RUNS)


def _build_nc(fbufs=2, hbufs=4, ps1bufs=7, ps2bufs=1, npb=NPB):
    nc = bacc.Bacc("TRN2", target_bir_lowering=False, debug=False,
                   num_devices=NCORES)
    # All inputs ship as ONE flat fp16 blob per core (layout in the module
    # header constants). fp16 matmul inputs keep the quantization noise
    # (~3e-4 std on the pre-clip scale logits) well inside the 2e-2
    # relative-error gate (measured 9.6e-3).
    blob = nc.dram_tensor("blob", (BLOB_N,), F16, kind="ExternalInput")
    # fp16 output: halves the result fetch through the tunnel; adds at most
    # 0.25 absolute rounding on values <= 1000.
    o = nc.dram_tensor("o", (6, PIX_PER_CORE), F16, kind="ExternalOutput")
    # per strip tensor id: (padded height, first shipped channel)
    sdim = {0: (PH_X, 0), 1: (PH_U, CU0)}

    with tile.TileContext(nc) as tc:
        with (
            tc.tile_pool(name="dpool", bufs=1, space="DRAM") as dpool,
            tc.tile_pool(name="wpool", bufs=1) as wpool,
            tc.tile_pool(name="cpool", bufs=1) as cpool,
            tc.tile_pool(name="fpool", bufs=fbufs) as fpool,
            tc.tile_pool(name="hpool", bufs=hbufs) as hpool,
            tc.tile_pool(name="spool", bufs=1) as spool,
            tc.tile_pool(name="opool", bufs=2) as opool,
            tc.tile_pool(name="ps1pool", bufs=ps1bufs, space="PSUM") as ps1pool,
            tc.tile_pool(name="ps2pool", bufs=ps2bufs, space="PSUM") as ps2pool,
        ):
            # --- W1 AllGather: shard (184, 2048) per core -> full (1472, 2048)
            # (copied to a dpool scratch tile first: EFA CC buffers need 4K
            # alignment, which a mid-blob offset can't guarantee)
            w1_cc_in = dpool.tile([W1SH, HID], F16)
            nc.sync.dma_start(w1_cc_in[:],
                              bass.AP(blob, W1_OFF, [[HID, W1SH], [1, HID]]))
            w1_full = dpool.tile([W1ROWS, HID], F16, addr_space="Shared")
            nc.gpsimd.collective_compute(
                "AllGather",
                mybir.AluOpType.bypass,
                replica_groups=[list(range(NCORES))],
                ins=[w1_cc_in[:].opt()],
                outs=[w1_full[:].opt()],
            )
            # --- W2 AllGather: 1536 floats per core -> flat (2048 x 6)
            w2_cc_in = dpool.tile([W2SH], F16)
            nc.sync.dma_start(w2_cc_in[:], bass.AP(blob, W2_OFF, [[1, W2SH]]))
            w2_full = dpool.tile([HID * 6], F16, addr_space="Shared")
            nc.gpsimd.collective_compute(
                "AllGather",
                mybir.AluOpType.bypass,
                replica_groups=[list(range(NCORES))],
                ins=[w2_cc_in[:].opt()],
                outs=[w2_full[:].opt()],
            )

            # --- rebuild the halo-padded strips from the raw (unpadded,
            # halo-less) shipped strips.
            xs_pad = dpool.tile([C, PH_X, PW], F16)
            us_pad = dpool.tile([CU, PH_U, PW], F16)
            # contiguous per-core halo contribution:
            # [x rows 29:32 | ups rows 29:32 | ups rows 0:3]
            hc = dpool.tile([HC_N], F16)
            nc.sync.dma_start(
                bass.AP(hc.tensor, hc.offset, [[PAD * W, C], [1, PAD * W]]),
                bass.AP(blob, XRAW_OFF + 29 * W,
                        [[ROWS_PER_CORE * W, C], [1, PAD * W]]))
            nc.sync.dma_start(
                bass.AP(hc.tensor, hc.offset + HX_N,
                        [[PAD * W, CU], [1, PAD * W]]),
                bass.AP(blob, URAW_OFF + 29 * W,
                        [[ROWS_PER_CORE * W, CU], [1, PAD * W]]))
            nc.sync.dma_start(
                bass.AP(hc.tensor, hc.offset + HX_N + HUB_N,
                        [[PAD * W, CU], [1, PAD * W]]),
                bass.AP(blob, URAW_OFF,
                        [[ROWS_PER_CORE * W, CU], [1, PAD * W]]))
            hall = dpool.tile([NCORES * HC_N], F16, addr_space="Shared")
            nc.gpsimd.collective_compute(
                "AllGather",
                mybir.AluOpType.bypass,
                replica_groups=[list(range(NCORES))],
                ins=[hc[:].opt()],
                outs=[hall[:].opt()],
            )

            # zero-fill both padded strips (covers pad columns and the image
            # border rows on the edge cores)
            zt = cpool.tile([30, PH_U * PW], F16)
            nc.any.memset(zt[:], 0.0)
            nc.sync.dma_start(
                bass.AP(xs_pad.tensor, xs_pad.offset,
                        [[PH_X * PW, C], [1, PH_X * PW]]),
                zt[0:C, 0:PH_X * PW])
            nc.sync.dma_start(
                bass.AP(us_pad.tensor, us_pad.offset,
                        [[PH_U * PW, CU], [1, PH_U * PW]]),
                zt[0:CU, 0:PH_U * PW])
            # scatter own rows into the interior [rows 3:35, cols 3:259]
            nc.sync.dma_start(
                bass.AP(xs_pad.tensor, xs_pad.offset + 3 * PW + 3,
                        [[PH_X * PW, C], [PW, ROWS_PER_CORE], [1, W]]),
                bass.AP(blob, XRAW_OFF,
                        [[ROWS_PER_CORE * W, C], [W, ROWS_PER_CORE], [1, W]]))
            nc.sync.dma_start(
                bass.AP(us_pad.tensor, us_pad.offset + 3 * PW + 3,
                        [[PH_U * PW, CU], [PW, ROWS_PER_CORE], [1, W]]),
                bass.AP(blob, URAW_OFF,
                        [[ROWS_PER_CORE * W, CU], [W, ROWS_PER_CORE], [1, W]]))

            # pick this core's neighbor slabs out of the gathered halo with
            # one-hot weights (selp = prev core, seln = next core; all-zero
            # on the edges) — ACT per-partition scale + DVE adds, SPMD-safe.
            sel_sb = cpool.tile([128, 16], F32)
            nc.sync.dma_start(sel_sb[:],
                              bass.AP(blob, SEL_OFF,
                                      [[32, 128], [1, 32]]).bitcast(F32))
            # partition counts chosen so the inner dim is one image row (256)
            # and the write-back DMAs balance against the strip layout
            hx_sb = cpool.tile([HX_N // W, NCORES, W], F16)       # [90, 8, 256]
            nc.sync.dma_start(
                hx_sb[:],
                bass.AP(hall.tensor, hall.offset,
                        [[W, HX_N // W], [HC_N, NCORES], [1, W]]))
            hu_sb = cpool.tile([HUB_N // W, NCORES, 2, W], F16)   # [48, 8, 2, 256]
            for t in range(2):
                nc.sync.dma_start(
                    hu_sb[:, :, t, :],
                    bass.AP(hall.tensor, hall.offset + HX_N + t * HUB_N,
                            [[W, HUB_N // W], [HC_N, NCORES], [1, W]]))

            def onehot_pick(src_j, sel_col, shape, tag):
                # sum_j sel[j] * src_j(j): ACT scale-mult then fused DVE MACs
                npart = shape[0]
                acc = spool.tile(shape, F16, tag=f"{tag}0")
                nc.scalar.activation(
                    acc[:], src_j(0),
                    mybir.ActivationFunctionType.Identity,
                    scale=sel_sb[0:npart, sel_col:sel_col + 1])
                for j in range(1, NCORES):
                    nxt = spool.tile(shape, F16, tag=f"{tag}{j}")
                    nc.vector.scalar_tensor_tensor(
                        nxt[:], src_j(j),
                        sel_sb[0:npart, sel_col + j:sel_col + j + 1], acc[:],
                        op0=mybir.AluOpType.mult,
                        op1=mybir.AluOpType.add)
                    acc = nxt
                return acc

            xtop = onehot_pick(lambda j: hx_sb[:, j, :], 0,
                               [HX_N // W, W], "hx")
            utop = onehot_pick(lambda j: hu_sb[:, j, 0, :], 0,
                               [HUB_N // W, W], "hut")
            ubot = onehot_pick(lambda j: hu_sb[:, j, 1, :], 8,
                               [HUB_N // W, W], "hub")
            # write the selected slabs into the halo rows [0:3] / [35:38],
            # cols 3:259 (flat element order matches the SBUF layout)
            nc.sync.dma_start(
                bass.AP(xs_pad.tensor, xs_pad.offset + 3,
                        [[PH_X * PW, C], [PW, PAD], [1, W]]),
                xtop[:])
            nc.sync.dma_start(
                bass.AP(us_pad.tensor, us_pad.offset + 3,
                        [[PH_U * PW, CU], [PW, PAD], [1, W]]),
                utop[:])
            nc.sync.dma_start(
                bass.AP(us_pad.tensor, us_pad.offset + 35 * PW + 3,
                        [[PH_U * PW, CU], [PW, PAD], [1, W]]),
                ubot[:])
            strip = {0: xs_pad, 1: us_pad}

            w1_sb = wpool.tile([128, NKG, HID], F16)

            # m-major lazy load: chunk m (all K-groups, one 128-wide hidden
            # block, 736KB) is issued just before pixel-block 0 consumes it,
            # so the matmul stream pipelines against the AllGather instead of
            # waiting for the whole 12MB. Gathered row g*128+p; the last
            # K-group only has 64 valid rows (1408..1471), so it loads as a
            # separate piece.
            def load_w1_chunk(m):
                base = m * 128
                nc.sync.dma_start(
                    w1_sb[:, 0:11, m * 128:(m + 1) * 128],
                    bass.AP(w1_full.tensor, w1_full.offset + base,
                            [[HID, 128], [128 * HID, 11], [1, 128]]))
                nc.sync.dma_start(
                    w1_sb[0:64, 11, m * 128:(m + 1) * 128],
                    bass.AP(w1_full.tensor,
                            w1_full.offset + 11 * 128 * HID + base,
                            [[HID, 64], [1, 128]]))
            # chunk stride padded to 16 fp16 so each lhsT slice is 32B-aligned
            w2_sb = wpool.tile([128, NM, 16], F16)
            nc.sync.dma_start(w2_sb[:, :, 0:6],
                              bass.AP(w2_full.tensor, w2_full.offset,
                                      [[6, 128], [768, NM], [1, 6]]))
            b3_sb = cpool.tile([6, 6], F32)
            nc.sync.dma_start(b3_sb[:],
                              bass.AP(blob, B3_OFF,
                                      [[12, 6], [1, 12]]).bitcast(F32))
            zb = cpool.tile([128, 1], F32)
            nc.any.memset(zb[:], 0.0)

            for pb in range(npb):
                R = 2 * pb  # first image row (strip-local) of this block
                # free layout per K-row: [2 rows, 264 cols] — a single
                # contiguous 520-element DRAM read covers both rows (the 8
                # inter-row pad columns land in [*, 0, 256:264] and are never
                # read by the matmuls).
                feat = fpool.tile([128, NKG, 2, 264], F16)
                for pc in _PIECES:
                    t = strip[pc["tid"]]
                    ph, cbase = sdim[pc["tid"]]
                    off = (t.offset + (pc["c0"] - cbase) * ph * PW
                           + (R + pc["di"]) * PW + pc["dj"])
                    src = bass.AP(
                        t.tensor,
                        off,
                        [[1, pc["ntap"]], [ph * PW, pc["nch"]], [1, 520]],
                    )
                    npart = pc["ntap"] * pc["nch"]
                    # dst: partitions p..p+npart, contiguous 520-elem span
                    # starting at [g, 0, 0]
                    dst = feat[pc["p"]:pc["p"] + npart, pc["g"], :, :]
                    dst = bass.AP(dst.tensor, dst.offset,
                                  [list(dst.ap[0]), [1, 520]])
                    nc.sync.dma_start(dst, src)
                # bias feature row: constant 1.0 (W1 row 1408 = b1)
                brow = feat[0:1, NKG - 1, :, :]
                brow = bass.AP(brow.tensor, brow.offset,
                               [list(brow.ap[0]), [1, 520]])
                nc.scalar.dma_start(brow,
                                    bass.AP(blob, ONES_OFF,
                                            [[520, 1], [1, 520]]))

                ps2 = ps2pool.tile([6, 512], F32)
                for m in range(NM):
                    if pb == 0:
                        load_w1_chunk(m)
                    ps = ps1pool.tile([128, 512], F32)
                    for g in range(NKG):
                        kg = 128 if g < NKG - 1 else KG_LAST
                        nc.tensor.matmul(
                            ps[:],
                            w1_sb[0:kg, g, m * 128:(m + 1) * 128],
                            feat[0:kg, g, :, 0:256],
                            start=(g == 0),
                            stop=(g == NKG - 1),
                        )
                    h = hpool.tile([128, 512], F16)
                    nc.scalar.activation(
                        h[:], ps[:], mybir.ActivationFunctionType.Relu,
                        bias=zb[:],
                    )
                    nc.tensor.matmul(
                        ps2[:],
                        w2_sb[:, m, 0:6],
                        h[:],
                        start=(m == 0),
                        stop=(m == NM - 1),
                        skip_group_check=True,
                    )

                # All transform ops run on partitions 0:6 with per-partition
                # scale/bias vectors; rows that don't apply get neutral values
                # (scale 0, bias 1) so every lane stays finite.
                # mu rows 0:3: (raw + b2_mu)*255 ; d rows 3:6: 1.1-(raw+b2_s)
                outm = spool.tile([6, 512], F16, tag="outm")
                nc.scalar.activation(
                    outm[:], ps2[:],
                    mybir.ActivationFunctionType.Identity,
                    bias=b3_sb[:, 1:2], scale=b3_sb[:, 0:1],
                )
                d = spool.tile([6, 512], F32, tag="d")
                nc.scalar.activation(
                    d[:], ps2[:],
                    mybir.ActivationFunctionType.Identity,
                    bias=b3_sb[:, 3:4], scale=b3_sb[:, 2:3],
                )
                r = spool.tile([6, 512], F32, tag="r")
                nc.vector.reciprocal(r[:], d[:])
                # n rows 3:6: 100*(raw + b2_s)
                n = spool.tile([6, 512], F32, tag="n")
                nc.scalar.activation(
                    n[:], ps2[:],
                    mybir.ActivationFunctionType.Identity,
                    bias=b3_sb[:, 5:6], scale=b3_sb[:, 4:5],
                )
                sc = spool.tile([6, 512], F32, tag="sc")
                nc.vector.tensor_mul(sc[:], n[:], r[:])
                scc = spool.tile([6, 512], F16, tag="scc")
                nc.vector.tensor_scalar(
                    scc[:], sc[:], 1000.0, 1e-8,
                    op0=mybir.AluOpType.min, op1=mybir.AluOpType.max,
                )
                pbs = slice(pb * 512, (pb + 1) * 512)
                nc.scalar.dma_start(o.ap()[0:3, pbs], outm[0:3, :])
                nc.scalar.dma_start(o.ap()[3:6, pbs], scc[3:6, :])

    nc.compile()
    return nc


_NC_CACHE = None


def _get_nc():
    global _NC_CACHE
    if _NC_CACHE is None:
        _NC_CACHE = _build_nc()
    return _NC_CACHE


_EXEC_CACHE = None


def _get_exec():
    """Compile the PJRT executable for the Bass kernel ONCE per process.

    run_bass_kernel_spmd builds a fresh jax.jit(shard_map(...)) every call,
    which re-traces and re-lowers (~80 ms) before the XLA disk-cache hit.
    This is the same execution path (bass_exec custom call on the 8 axon
    NeuronCores) with the compiled callable cached, and with the
    ExternalOutput staging buffers kept device-resident (they are not
    donated, so one device_put serves every call) instead of shipping host
    zero arrays through the tunnel each time.
    """
    global _EXEC_CACHE
    if _EXEC_CACHE is not None:
        return _EXEC_CACHE

    import jax
    from jax.sharding import Mesh, PartitionSpec
    from jax.experimental.shard_map import shard_map

    nc = _get_nc()
    bass2jax.install_neuronx_cc_hook()
    partition_name = (nc.partition_id_tensor.name
                     if nc.partition_id_tensor else None)
    in_names: list[str] = []
    out_names: list[str] = []
    out_avals = []
    in_shapes = []
    for alloc in nc.m.functions[0].allocations:
        if not isinstance(alloc, mybir.MemoryLocationSet):
            continue
        name = alloc.memorylocations[0].name
        if alloc.kind == "ExternalInput":
            if name != partition_name:
                in_names.append(name)
                in_shapes.append((tuple(alloc.tensor_shape),
                                  mybir.dt.np(alloc.dtype)))
        elif alloc.kind == "ExternalOutput":
            out_names.append(name)
            out_avals.append(jax.core.ShapedArray(
                tuple(alloc.tensor_shape), mybir.dt.np(alloc.dtype)))
    n_params = len(in_names)
    bind_names = tuple(in_names) + tuple(out_names) + (
        (partition_name,) if partition_name else ())

    def _body(*args):
        # trailing args are the ExternalOutput staging buffers — the hook
        # requires every bass_exec operand to be a literal jit parameter
        operands = list(args)
        if partition_name is not None:
            operands.append(bass2jax.partition_id_tensor())
        return tuple(bass2jax._bass_exec_p.bind(
            *operands,
            out_avals=tuple(out_avals),
            in_names=bind_names,
            out_names=tuple(out_names),
            lowering_input_output_aliases=(),
            sim_require_finite=True,
            sim_require_nnan=True,
            nc=nc,
        ))

    devices = jax.devices()[:NCORES]
    mesh = Mesh(np.asarray(devices), ("core",))
    n_outs = len(out_names)
    sharded = jax.jit(shard_map(
        _body, mesh=mesh,
        in_specs=(PartitionSpec("core"),) * (n_params + n_outs),
        out_specs=(PartitionSpec("core"),) * n_outs,
        check_rep=False,
    ))
    arg_structs = [
        jax.ShapeDtypeStruct(
            (NCORES * shape[0], *shape[1:]) if shape else (NCORES,), dtype)
        for shape, dtype in in_shapes
    ] + [
        jax.ShapeDtypeStruct((NCORES * a.shape[0], *a.shape[1:]), a.dtype)
        for a in out_avals
    ]
    compiled = sharded.lower(*arg_structs).compile()
    # The staging buffers are NOT donated, so these device-resident zeros
    # survive across calls: one device_put here, zero tunnel bytes per call.
    # (o is fully overwritten by the kernel, so their value never matters.)
    from jax.sharding import NamedSharding
    zero_args = [
        jax.device_put(np.zeros((NCORES * a.shape[0], *a.shape[1:]), a.dtype),
                       NamedSharding(mesh, PartitionSpec("core")))
        for a in out_avals
    ]
    _EXEC_CACHE = (compiled, in_names, out_names, out_avals, zero_args)
    # warm up the dispatch path once so lazy PJRT/runtime init happens at
    # build time rather than inside the first real call
    try:
        warm = [np.zeros(s.shape, s.dtype) for s in arg_structs[:n_params]]
        for a in compiled(*warm, *zero_args):
            np.asarray(a)
    except Exception:
        pass
    return _EXEC_CACHE


def _run(gmap):
    """Execute on the 8 cores; gmap maps input name -> concatenated global
    array (cores stacked on axis 0). Returns {out name: (NCORES, ...) array}."""
    compiled, in_names, out_names, out_avals, zero_args = _get_exec()
    out_arrs = compiled(*[gmap[name] for name in in_names], *zero_args)
    return {
        name: np.asarray(a).reshape(NCORES, *out_avals[i].shape)
        for i, (name, a) in enumerate(zip(out_names, out_arrs))
    }


def _prep_host_inputs(x, x_ups, W1, b1, W2, b2):
    """Assemble the global input blob: (NCORES * BLOB_N,) fp16."""
    x = np.asarray(x)
    x_ups = np.asarray(x_ups)
    x16 = x[0].astype(np.float16)                  # (C, 256, 256)
    u16 = x_ups[0, CU0:].astype(np.float16)        # (CU, 256, 256)

    gblob = np.zeros((NCORES, BLOB_N), np.float16)
    xv = gblob[:, XRAW_OFF:XRAW_OFF + XRAW_N].reshape(
        NCORES, C, ROWS_PER_CORE, W)
    uv = gblob[:, URAW_OFF:URAW_OFF + URAW_N].reshape(
        NCORES, CU, ROWS_PER_CORE, W)
    for k in range(NCORES):
        r0 = k * ROWS_PER_CORE
        xv[k] = x16[:, r0:r0 + ROWS_PER_CORE]
        uv[k] = u16[:, r0:r0 + ROWS_PER_CORE]
    # one-hot halo selectors (f32, bitcast into the f16 blob), replicated
    # across the 128 SBUF partitions: cols 0:8 pick the prev core's slab,
    # cols 8:16 the next core's
    for k in range(NCORES):
        sel = gblob[k, SEL_OFF:SEL_OFF + 128 * 16 * 2].view(
            np.float32).reshape(128, 16)
        if k > 0:
            sel[:, k - 1] = 1.0
        if k < NCORES - 1:
            sel[:, 8 + k + 1] = 1.0

    # reordered W1; bias row (=b1) at position 1408; zero-pad to 1472 rows.
    # The concatenation of the 8 per-core 184-row shards IS the full
    # reordered matrix, written straight into the blob region.
    W1g = np.zeros((W1ROWS, HID), np.float16)
    W1p = np.asarray(W1)[_PERM]
    W1g[:1408] = W1p[:1408]
    W1g[1408] = np.asarray(b1)
    W1g[1409:1471] = W1p[1408:]
    gblob[:, W1_OFF:W1_OFF + W1_N] = W1g.reshape(NCORES, W1_N)
    # W2 column-reordered: [mu0 mu1 mu2 s0 s1 s2]; shards = flat slices
    W2a = np.asarray(W2).astype(np.float32)
    b2a = np.asarray(b2).astype(np.float32)
    W2f = np.ascontiguousarray(W2a[:, [0, 2, 4, 1, 3, 5]],
                               dtype=np.float16).ravel()
    gblob[:, W2_OFF:W2_OFF + W2SH] = W2f.reshape(NCORES, W2SH)
    gblob[:, ONES_OFF:ONES_OFF + 520] = np.float16(1.0)
    b3 = np.zeros((6, 6), np.float32)
    b3[0:3, 0] = 255.0                        # sA (mu scale)
    b3[0:3, 1] = b2a[[0, 2, 4]] * 255.0       # bA (mu bias)
    b3[3:6, 2] = -1.0                         # sC (d scale)
    b3[0:3, 3] = 1.0                          # bC neutral rows
    b3[3:6, 3] = 1.1 - b2a[[1, 3, 5]]         # bC (d bias)
    b3[3:6, 4] = 100.0                        # sD (n scale)
    b3[0:3, 5] = 1.0                          # bD neutral rows
    b3[3:6, 5] = 100.0 * b2a[[1, 3, 5]]       # bD (n bias)
    for k in range(NCORES):
        gblob[k, B3_OFF:B3_OFF + 72].view(np.float32)[:] = b3.ravel()

    return {"blob": gblob.reshape(-1)}


def kernel(x, x_ups, W1, b1, W2, b2):
    gmap = _prep_host_inputs(x, x_ups, W1, b1, W2, b2)
    res = _run(gmap)
    ocs = res["o"]                                                # (8, 6, 8192)
    flat = ocs.transpose(0, 2, 1).reshape(H * W, 6)               # (65536, 6)
    out = flat.reshape(H * W, 2, 3).transpose(0, 2, 1)            # (65536, 3, 2)
    return np.ascontiguousarray(out[None]).astype(np.float32)     # (1, 65536, 3, 2)

